# revision 24
# baseline (speedup 1.0000x reference)
"""Trainium2 Bass kernel for the JaCDE dense-MLP vector-field problem.

Math: h_dot = sum_{k=0..8} M^k jx with M v = dtanh . (wout @ (drelu . (wh @ v)))
and jx = dtanh . (wout @ (drelu . (wx @ xdot))) -- O(B*H^2) matmuls instead of
the reference's O(B*H^3) d_outer contraction.

Schedule: activations live transposed [H, B_local]; H splits into 2 partition
tiles (m) and the local batch into 2 column sub-blocks (A/B) that form two
independent dependency chains, so PE / DVE / ACT overlap across sub-blocks
instead of serializing per recurrence step.

Precision: phase 1 (the l1 relu mask) runs full f32 so the hard gate is
computed from exact l1 (rounded weights there flip gates: 1.4e-1 error).  The
9 recurrence applications run bf16 (weights + activations): smooth rounding
only, measured 4.8e-3 end-to-end vs the 2e-2 budget.  bf16 halves DVE cost
(2x mode) and keeps fp32 PSUM accumulation.

h_dot accumulation: identity matmuls add each term into a dedicated PSUM bank
(ZACC), freeing DVE/GpSimd (GpSimd shares an SBUF port with DVE) from the
critical path.  One ACT copy drains ZACC at the end.

Sub-block chunk layout for [128, 4, SB] tiles: chunk c = 2*sub + m, i.e.
[m0A | m1A | m0B | m1B]; chunk pairs 0:2 / 2:4 are the A / B halves and per-m
slices are [:, m::2, :].

Sharding: pure data parallel, batch 2048 -> 8 cores x 256.
"""

import contextlib

import numpy as np
import ml_dtypes

import concourse.tile as tile
from concourse import bacc, mybir
from concourse.bass_utils import run_bass_kernel_spmd

B, H, IN = 2048, 256, 64
K_TERMS = 8
N_CORES = 8
BL = B // N_CORES  # 256 batch rows per core
HH = H // 2        # 128, H partition halves
SB = BL // 2       # 128, batch sub-block columns

f32 = mybir.dt.float32
f32r = mybir.dt.float32r
bf16 = mybir.dt.bfloat16

_ALU = mybir.AluOpType
_ACT = mybir.ActivationFunctionType

N_WARMUP_MM = 6  # ~1.5us of PE work to lift the HAM clock gate during DMAs


def _build(repeat=1, loop=0):
    nc = bacc.Bacc(None, target_bir_lowering=False)

    # Inputs arrive as host-packed byte blobs (one DMA each: rows of
    # contiguous bytes -> minimal descriptors), ordered by phase-1 need.
    # blobA1 row p: whT[p]     | hT[p]       (2048 B, f32)
    # blobA2 row p: whT[128+p] | hT[128+p]   (2048 B, f32)
    # blobB  row p: whB[p] | whB[128+p] | woB[p] | woB[128+p] | idB[p]  (2304 B, bf16)
    # blob64 row p: wxT[p] | xT[p] | wxB[p] | xdB[p]     (3072 B)
    u8 = mybir.dt.uint8
    blobA1d = nc.dram_tensor("blobA1", [HH, 2048], u8, kind="ExternalInput")
    blobA2d = nc.dram_tensor("blobA2", [HH, 2048], u8, kind="ExternalInput")
    blobBd = nc.dram_tensor("blobB", [HH, 2304], u8, kind="ExternalInput")
    blob64d = nc.dram_tensor("blob64", [IN, 3072], u8, kind="ExternalInput")
    biasd = nc.dram_tensor("biasb", [HH, 16], u8, kind="ExternalInput")
    hdT = nc.dram_tensor("hdT", [H, BL], f32, kind="ExternalOutput")

    with tile.TileContext(nc) as tc:
        with (
            tc.tile_pool(name="wpool", bufs=1) as wp,
            tc.tile_pool(name="apool", bufs=1) as apool,
            tc.tile_pool(name="rot", bufs=4) as rot,
            tc.tile_pool(name="ps", bufs=1, space="PSUM") as ps,
        ):
            # ---- blob inputs to SBUF (4 DMAs, ordered by phase-1 need:
            # blob64 (wx/x, small) first, then whT/hT, then bf16 weights) ----
            blA1 = wp.tile([HH, 2048], u8, tag="blA1")
            blA2 = wp.tile([HH, 2048], u8, tag="blA2")
            blB = wp.tile([HH, 2304], u8, tag="blB")
            bl64 = wp.tile([IN, 3072], u8, tag="bl64")
            bl_b = wp.tile([HH, 16], u8, tag="blb")
            nc.sync.dma_start(bl64[:], blob64d[:])
            nc.sync.dma_start(blA1[:], blobA1d[:])
            nc.sync.dma_start(blA2[:], blobA2d[:])
            nc.scalar.dma_start(blB[:], blobBd[:])
            nc.scalar.dma_start(bl_b[:], biasd[:])
            wh_r = [blA1[:, 0:1024].bitcast(f32), blA2[:, 0:1024].bitcast(f32)]
            h_sb = [blA1[:, 1024:2048].bitcast(f32), blA2[:, 1024:2048].bitcast(f32)]
            wh_b = [blB[:, 0:512].bitcast(bf16), blB[:, 512:1024].bitcast(bf16)]
            wo_b = [blB[:, 1024:1536].bitcast(bf16), blB[:, 1536:2048].bitcast(bf16)]
            id_b = blB[:, 2048:2304].bitcast(bf16)
            wx_r = bl64[:, 0:1024].bitcast(f32)
            x_sb = bl64[:, 1024:2048].bitcast(f32)
            wx_b = bl64[:, 2048:2560].bitcast(bf16)
            xd_b = bl64[:, 2560:3072].bitcast(bf16).rearrange("p (s c) -> p s c", c=SB)
            b0_sb = bl_b[:, 0:8].bitcast(f32)
            b1_sb = bl_b[:, 8:16].bitcast(f32)

            # ---- warmup: ACT table preload (Tanh) + dummy matmuls with no
            # data deps so the HAM clock gate opens while DMAs are in flight.
            wt = wp.tile([HH, 8], f32, tag="wt")
            nc.vector.memset(wt[:], 0.0)
            nc.scalar.activation(wt[:, 0:1], wt[:, 1:2], _ACT.Tanh)
            wu_w = wp.tile([HH, HH], bf16, tag="wu_w")
            wu_v = wp.tile([HH, BL], bf16, tag="wu_v")
            nc.vector.memset(wu_w[:].bitcast(f32), 0.0)
            nc.vector.memset(wu_v[:].bitcast(f32), 0.0)
            wu_p = ps.tile([HH, 4, SB], f32, tag="zacc", name="wu_p")
            for _ in range(N_WARMUP_MM):
                nc.tensor.matmul(
                    wu_p[:, 0:2, :], wu_w[:], wu_v[:], start=True, stop=True
                )

            loop_cm = tc.For_i(0, loop, 1) if loop else contextlib.nullcontext()
            with loop_cm:
             for _rep in range(repeat):
              # ---- phase 1: l1 = wx@x + wh@h + b0 (f32, full width) ----
              ph1 = [
                  ps.tile([HH, 2, SB], f32, tag=f"ph1_{m}", name=f"ph1_{m}")
                  for m in range(2)
              ]
              # relu doubles as the gate mask: (l1 > 0) == (relu(l1) > 0)
              # (bf16 rounding preserves sign/zero exactly), so no separate
              # l1 tile is needed.  Chunk layout [m0A|m1A|m0B|m1B].
              relu_arr = apool.tile([HH, 4, SB], bf16, tag="relu", name="relu_arr")
              for m in range(2):
                  mSl = slice(m * HH, (m + 1) * HH)
                  nc.tensor.matmul(
                      ph1[m][:], wx_r[:, mSl], x_sb[:, :], start=True, stop=False
                  )
              for m in range(2):
                  mSl = slice(m * HH, (m + 1) * HH)
                  nc.tensor.matmul(
                      ph1[m][:], wh_r[0][:, mSl], h_sb[0][:, :], start=False, stop=False
                  )
                  nc.tensor.matmul(
                      ph1[m][:], wh_r[1][:, mSl], h_sb[1][:, :], start=False, stop=True
                  )
              for m in range(2):
                  nc.scalar.activation(
                      relu_arr[:, m::2, :], ph1[m][:], _ACT.Relu,
                      bias=b0_sb[:, m : m + 1],
                  )

              # ---- phase 2: lout = wout@relu + b1; dtanh = 1 - tanh^2 ----
              ph2 = [
                  ps.tile([HH, 2, SB], f32, tag=f"pWO_{s}", name=f"ph2_{m}")
                  for m, s in ((0, "A"), (1, "B"))
              ]
              dtanh_arr = apool.tile([HH, 4, SB], bf16, tag="dt", name="dtanh_arr")
              for m in range(2):
                  mSl = slice(m * HH, (m + 1) * HH)
                  nc.tensor.matmul(
                      ph2[m][:], wo_b[0][:, mSl], relu_arr[:, 0::2, :],
                      start=True, stop=False,
                  )
                  nc.tensor.matmul(
                      ph2[m][:], wo_b[1][:, mSl], relu_arr[:, 1::2, :],
                      start=False, stop=True,
                  )
              for m in range(2):
                  tanh = apool.tile([HH, BL], f32, tag=f"tanh{m}", name=f"tanh{m}")
                  sq = apool.tile([HH, BL], f32, tag=f"sq{m}", name=f"sq{m}")
                  nc.scalar.activation(
                      tanh[:], ph2[m][:].rearrange("p a b -> p (a b)"), _ACT.Tanh,
                      bias=b1_sb[:, m : m + 1],
                  )
                  # dtanh = 1 - tanh^2 on DVE (slots into idle gaps between
                  # the iter-0 gates and the first dtanh-multiplies)
                  nc.vector.scalar_tensor_tensor(
                      sq[:], tanh[:], -1.0, tanh[:], _ALU.mult, _ALU.mult
                  )
                  nc.vector.tensor_scalar(
                      dtanh_arr[:, m::2, :],
                      sq[:].rearrange("p (s c) -> p s c", c=SB),
                      1.0, None, _ALU.add,
                  )

              # ---- phase 3 + recurrence: 9 applications of the chain
              #   stage1 (wx@xdot first, wh@curr after) -> gate -> wout -> dtanh-mul
              # two batch sub-blocks pipeline through PE/DVE/ACT.
              pWH = [
                  ps.tile([HH, 2, SB], f32, tag=f"pWH_{s}", name=f"pWH_{s}")
                  for s in ("A", "B")
              ]
              pWO = [
                  ps.tile([HH, 2, SB], f32, tag=f"pWO_{s}", name=f"pWO_{s}")
                  for s in ("A", "B")
              ]
              zacc = ps.tile([HH, 4, SB], f32, tag="zacc", name="zacc")

              # Software-pipelined recurrence: the A and B batch sub-blocks
              # are independent chains; B's stages are emitted half an
              # iteration behind A so each chain's cross-engine handoff
              # latency is covered by the other chain's work.  Identity
              # matmuls (ZACC accumulate) lag behind the next wh-stage so
              # the PE never waits on a dtanh-multiply.
              def stage1_mms(s, it, curr):
                  cSl = slice(2 * s, 2 * s + 2)
                  for m in range(2):
                      mSl = slice(m * HH, (m + 1) * HH)
                      if it == 0:
                          nc.tensor.matmul(
                              pWH[s][:, m, :], wx_b[:, mSl], xd_b[:, s, :],
                              start=True, stop=True,
                          )
                      else:
                          nc.tensor.matmul(
                              pWH[s][:, m, :], wh_b[0][:, mSl],
                              curr[:, 2 * s, :], start=True, stop=False,
                          )
                          nc.tensor.matmul(
                              pWH[s][:, m, :], wh_b[1][:, mSl],
                              curr[:, 2 * s + 1, :], start=False, stop=True,
                          )

              def gate(s, tg):
                  cSl = slice(2 * s, 2 * s + 2)
                  nc.vector.scalar_tensor_tensor(
                      tg[:, cSl, :], relu_arr[:, cSl, :], 0.0,
                      pWH[s][:], _ALU.is_gt, _ALU.mult,
                  )

              def wout_mms(s, tg):
                  for m in range(2):
                      mSl = slice(m * HH, (m + 1) * HH)
                      nc.tensor.matmul(
                          pWO[s][:, m, :], wo_b[0][:, mSl],
                          tg[:, 2 * s, :], start=True, stop=False,
                      )
                      nc.tensor.matmul(
                          pWO[s][:, m, :], wo_b[1][:, mSl],
                          tg[:, 2 * s + 1, :], start=False, stop=True,
                      )

              def mul(s, newc):
                  cSl = slice(2 * s, 2 * s + 2)
                  nc.vector.tensor_mul(
                      newc[:, cSl, :], dtanh_arr[:, cSl, :], pWO[s][:]
                  )

              def id_mm(s, it, newc):
                  nc.tensor.matmul(
                      zacc[:, 2 * s : 2 * s + 2, :], id_b[:, :],
                      newc[:, 2 * s : 2 * s + 2, :],
                      start=(it == 0 and s == 0), stop=(it == K_TERMS and s == 1),
                      skip_group_check=True,
                  )

              curr = None
              pend = []  # (s, it, tile) id-mms not yet emitted
              tiles = {}
              for it in range(K_TERMS + 1):
                  tiles[it] = (
                      rot.tile([HH, 4, SB], bf16, tag="tg", name="tg"),
                      rot.tile([HH, 4, SB], bf16, tag="curr", name="newc"),
                  )
                  tg, newc = tiles[it]
                  for s in range(2):
                      stage1_mms(s, it, curr)
                      gate(s, tg)
                      wout_mms(s, tg)
                      while pend:
                          id_mm(*pend.pop(0))
                      mul(s, newc)
                      pend.append((s, it, newc))
                  curr = newc
              while pend:
                  id_mm(*pend.pop(0))

              # ---- tail: drain ZACC (split DVE/ACT) and store (2 DMAs) ----
              hdot_sb = apool.tile([HH, 4, SB], f32, tag="hd", name="hdot_sb")
              nc.vector.tensor_copy(hdot_sb[:, 0::2, :], zacc[:, 0::2, :])
              nc.scalar.copy(hdot_sb[:, 1::2, :], zacc[:, 1::2, :])
              nc.sync.dma_start(hdT[0:HH, :], hdot_sb[:, 0::2, :])
              nc.scalar.dma_start(hdT[HH:H, :], hdot_sb[:, 1::2, :])

    nc.compile()
    return nc


_NC = {}


def _get_nc(repeat=1, loop=0):
    key = (repeat, loop)
    if key not in _NC:
        _NC[key] = _build(repeat, loop)
    return _NC[key]


def make_in_maps(h, x, xdot, wx, wh, wout, b0, b1):
    h = np.asarray(h, np.float32)
    x = np.asarray(x, np.float32)
    xdot = np.asarray(xdot, np.float32)
    wx = np.asarray(wx, np.float32)
    wh = np.asarray(wh, np.float32)
    wout = np.asarray(wout, np.float32)
    b0 = np.asarray(b0, np.float32)
    b1 = np.asarray(b1, np.float32)

    whT = np.ascontiguousarray(wh.T)
    woT = np.ascontiguousarray(wout.T)
    wxT = np.ascontiguousarray(wx.T)
    whB = whT.astype(ml_dtypes.bfloat16)
    woB = woT.astype(ml_dtypes.bfloat16)
    wxB = wxT.astype(ml_dtypes.bfloat16)
    idB = np.eye(HH, dtype=np.float32).astype(ml_dtypes.bfloat16)
    b0c = np.ascontiguousarray(np.stack([b0[:HH], b0[HH:]], axis=1))
    b1c = np.ascontiguousarray(np.stack([b1[:HH], b1[HH:]], axis=1))

    u8 = np.uint8
    w_part = [whT[:HH].view(u8), whT[HH:].view(u8)]
    blobB = np.concatenate(
        [whB[:HH].view(u8), whB[HH:].view(u8),
         woB[:HH].view(u8), woB[HH:].view(u8), idB.view(u8)], axis=1
    )
    bias_blob = np.concatenate([b0c.view(u8), b1c.view(u8)], axis=1)

    in_maps = []
    for i in range(N_CORES):
        sl = slice(i * BL, (i + 1) * BL)
        hT = np.ascontiguousarray(h[sl].T)
        xT = np.ascontiguousarray(x[sl].T)
        xdB = np.ascontiguousarray(xdot[sl].T).astype(ml_dtypes.bfloat16)
        blobA1 = np.concatenate([w_part[0], hT[:HH].view(u8)], axis=1)
        blobA2 = np.concatenate([w_part[1], hT[HH:].view(u8)], axis=1)
        blob64 = np.concatenate(
            [wxT.view(u8), xT.view(u8), wxB.view(u8), xdB.view(u8)], axis=1
        )
        in_maps.append(
            {
                "blobA1": blobA1,
                "blobA2": blobA2,
                "blobB": blobB,
                "blob64": blob64,
                "biasb": bias_blob,
            }
        )
    return in_maps


def kernel(h, x, xdot, wx, wh, wout, b0, b1):
    in_maps = make_in_maps(h, x, xdot, wx, wh, wout, b0, b1)
    res = run_bass_kernel_spmd(_get_nc(), in_maps, core_ids=list(range(N_CORES)))
    out = np.empty((B, H), np.float32)
    for i in range(N_CORES):
        out[i * BL : (i + 1) * BL] = res.results[i]["hdT"].T
    return out


# revision 25
# speedup vs baseline: 1.0278x; 1.0278x over previous
"""Trainium2 Bass kernel for the JaCDE dense-MLP vector-field problem.

Math: h_dot = sum_{k=0..8} M^k jx with M v = dtanh . (wout @ (drelu . (wh @ v)))
and jx = dtanh . (wout @ (drelu . (wx @ xdot))) -- O(B*H^2) matmuls instead of
the reference's O(B*H^3) d_outer contraction.

Schedule: activations live transposed [H, B_local]; H splits into 2 partition
tiles (m) and the local batch into 2 column sub-blocks (A/B) that form two
independent dependency chains, so PE / DVE / ACT overlap across sub-blocks
instead of serializing per recurrence step.

Precision: phase 1 (the l1 relu mask) runs full f32 so the hard gate is
computed from exact l1 (rounded weights there flip gates: 1.4e-1 error).  The
9 recurrence applications run bf16 (weights + activations): smooth rounding
only, measured 4.8e-3 end-to-end vs the 2e-2 budget.  bf16 halves DVE cost
(2x mode) and keeps fp32 PSUM accumulation.

h_dot accumulation: identity matmuls add each term into a dedicated PSUM bank
(ZACC), freeing DVE/GpSimd (GpSimd shares an SBUF port with DVE) from the
critical path.  One ACT copy drains ZACC at the end.

Sub-block chunk layout for [128, 4, SB] tiles: chunk c = 2*sub + m, i.e.
[m0A | m1A | m0B | m1B]; chunk pairs 0:2 / 2:4 are the A / B halves and per-m
slices are [:, m::2, :].

Sharding: pure data parallel, batch 2048 -> 8 cores x 256.
"""

import contextlib

import numpy as np
import ml_dtypes

import concourse.tile as tile
from concourse import bacc, mybir
from concourse.bass_utils import run_bass_kernel_spmd

B, H, IN = 2048, 256, 64
K_TERMS = 8
N_CORES = 8
BL = B // N_CORES  # 256 batch rows per core
HH = H // 2        # 128, H partition halves
SB = BL // 2       # 128, batch sub-block columns

f32 = mybir.dt.float32
f32r = mybir.dt.float32r
bf16 = mybir.dt.bfloat16

_ALU = mybir.AluOpType
_ACT = mybir.ActivationFunctionType

N_WARMUP_MM = 6  # ~1.5us of PE work to lift the HAM clock gate during DMAs


def _build(repeat=1, loop=0):
    nc = bacc.Bacc(None, target_bir_lowering=False)

    # Inputs arrive as host-packed byte blobs (one DMA each: rows of
    # contiguous bytes -> minimal descriptors), ordered by phase-1 need.
    # blobA1 row p: whT[p]     | hT[p]       (2048 B, f32)
    # blobA2 row p: whT[128+p] | hT[128+p]   (2048 B, f32)
    # blobB  row p: whB[p] | whB[128+p] | woB[p] | woB[128+p] | idB[p]  (2304 B, bf16)
    # blob64 row p: wxT[p] | xT[p] | wxB[p] | xdB[p]     (3072 B)
    u8 = mybir.dt.uint8
    blobA1d = nc.dram_tensor("blobA1", [HH, 2048], u8, kind="ExternalInput")
    blobA2d = nc.dram_tensor("blobA2", [HH, 2048], u8, kind="ExternalInput")
    blobBd = nc.dram_tensor("blobB", [HH, 2304], u8, kind="ExternalInput")
    blob64d = nc.dram_tensor("blob64", [IN, 3072], u8, kind="ExternalInput")
    biasd = nc.dram_tensor("biasb", [HH, 16], u8, kind="ExternalInput")
    hdT = nc.dram_tensor("hdT", [H, BL], f32, kind="ExternalOutput")

    with tile.TileContext(nc) as tc:
        with (
            tc.tile_pool(name="wpool", bufs=1) as wp,
            tc.tile_pool(name="apool", bufs=1) as apool,
            tc.tile_pool(name="rot", bufs=3) as rot,
            tc.tile_pool(name="ps", bufs=1, space="PSUM") as ps,
        ):
            # ---- blob inputs to SBUF (4 DMAs, ordered by phase-1 need:
            # blob64 (wx/x, small) first, then whT/hT, then bf16 weights) ----
            blA1 = wp.tile([HH, 2048], u8, tag="blA1")
            blA2 = wp.tile([HH, 2048], u8, tag="blA2")
            blB = wp.tile([HH, 2304], u8, tag="blB")
            bl64 = wp.tile([IN, 3072], u8, tag="bl64")
            bl_b = wp.tile([HH, 16], u8, tag="blb")
            nc.sync.dma_start(bl64[:], blob64d[:])
            nc.sync.dma_start(blA1[:], blobA1d[:])
            nc.sync.dma_start(blA2[:], blobA2d[:])
            nc.scalar.dma_start(blB[:], blobBd[:])
            nc.scalar.dma_start(bl_b[:], biasd[:])
            wh_r = [blA1[:, 0:1024].bitcast(f32), blA2[:, 0:1024].bitcast(f32)]
            h_sb = [blA1[:, 1024:2048].bitcast(f32), blA2[:, 1024:2048].bitcast(f32)]
            wh_b = [blB[:, 0:512].bitcast(bf16), blB[:, 512:1024].bitcast(bf16)]
            wo_b = [blB[:, 1024:1536].bitcast(bf16), blB[:, 1536:2048].bitcast(bf16)]
            id_b = blB[:, 2048:2304].bitcast(bf16)
            wx_r = bl64[:, 0:1024].bitcast(f32)
            x_sb = bl64[:, 1024:2048].bitcast(f32)
            wx_b = bl64[:, 2048:2560].bitcast(bf16)
            xd_b = bl64[:, 2560:3072].bitcast(bf16).rearrange("p (s c) -> p s c", c=SB)
            b0_sb = bl_b[:, 0:8].bitcast(f32)
            b1_sb = bl_b[:, 8:16].bitcast(f32)

            # ---- warmup: ACT table preload (Tanh) + dummy matmuls with no
            # data deps so the HAM clock gate opens while DMAs are in flight.
            wt = wp.tile([HH, 8], f32, tag="wt")
            nc.vector.memset(wt[:], 0.0)
            nc.scalar.activation(wt[:, 0:1], wt[:, 1:2], _ACT.Tanh)
            wu_w = wp.tile([HH, HH], bf16, tag="wu_w")
            wu_v = wp.tile([HH, BL], bf16, tag="wu_v")
            nc.vector.memset(wu_w[:].bitcast(f32), 0.0)
            nc.vector.memset(wu_v[:].bitcast(f32), 0.0)
            wu_p = ps.tile([HH, 4, SB], f32, tag="zacc", name="wu_p")
            for _ in range(N_WARMUP_MM):
                nc.tensor.matmul(
                    wu_p[:, 0:2, :], wu_w[:], wu_v[:], start=True, stop=True
                )

            loop_cm = tc.For_i(0, loop, 1) if loop else contextlib.nullcontext()
            with loop_cm:
             for _rep in range(repeat):
              # ---- phase 1: l1 = wx@x + wh@h + b0 (f32, full width) ----
              ph1 = [
                  ps.tile([HH, 2, SB], f32, tag=f"ph1_{m}", name=f"ph1_{m}")
                  for m in range(2)
              ]
              # relu doubles as the gate mask: (l1 > 0) == (relu(l1) > 0)
              # (bf16 rounding preserves sign/zero exactly), so no separate
              # l1 tile is needed.  Chunk layout [m0A|m1A|m0B|m1B].
              relu_arr = apool.tile([HH, 4, SB], bf16, tag="relu", name="relu_arr")
              for m in range(2):
                  mSl = slice(m * HH, (m + 1) * HH)
                  nc.tensor.matmul(
                      ph1[m][:], wx_r[:, mSl], x_sb[:, :], start=True, stop=False
                  )
              for m in range(2):
                  mSl = slice(m * HH, (m + 1) * HH)
                  nc.tensor.matmul(
                      ph1[m][:], wh_r[0][:, mSl], h_sb[0][:, :], start=False, stop=False
                  )
                  nc.tensor.matmul(
                      ph1[m][:], wh_r[1][:, mSl], h_sb[1][:, :], start=False, stop=True
                  )
              for m in range(2):
                  nc.scalar.activation(
                      relu_arr[:, m::2, :], ph1[m][:], _ACT.Relu,
                      bias=b0_sb[:, m : m + 1],
                  )

              # ---- phase 2: lout = wout@relu + b1; dtanh = 1 - tanh^2 ----
              ph2 = [
                  ps.tile([HH, 2, SB], f32, tag=f"pWO_{s}", name=f"ph2_{m}")
                  for m, s in ((0, "A"), (1, "B"))
              ]
              dtanh_arr = apool.tile([HH, 4, SB], bf16, tag="dt", name="dtanh_arr")
              for m in range(2):
                  mSl = slice(m * HH, (m + 1) * HH)
                  nc.tensor.matmul(
                      ph2[m][:], wo_b[0][:, mSl], relu_arr[:, 0::2, :],
                      start=True, stop=False,
                  )
                  nc.tensor.matmul(
                      ph2[m][:], wo_b[1][:, mSl], relu_arr[:, 1::2, :],
                      start=False, stop=True,
                  )
              for m in range(2):
                  tanh = apool.tile([HH, BL], f32, tag=f"tanh{m}", name=f"tanh{m}")
                  sq = apool.tile([HH, BL], f32, tag=f"sq{m}", name=f"sq{m}")
                  nc.scalar.activation(
                      tanh[:], ph2[m][:].rearrange("p a b -> p (a b)"), _ACT.Tanh,
                      bias=b1_sb[:, m : m + 1],
                  )
                  # dtanh = 1 - tanh^2 on DVE (slots into idle gaps between
                  # the iter-0 gates and the first dtanh-multiplies)
                  nc.vector.scalar_tensor_tensor(
                      sq[:], tanh[:], -1.0, tanh[:], _ALU.mult, _ALU.mult
                  )
                  nc.vector.tensor_scalar(
                      dtanh_arr[:, m::2, :],
                      sq[:].rearrange("p (s c) -> p s c", c=SB),
                      1.0, None, _ALU.add,
                  )

              # ---- phase 3 + recurrence: 9 applications of the chain
              #   stage1 (wx@xdot first, wh@curr after) -> gate -> wout -> dtanh-mul
              # two batch sub-blocks pipeline through PE/DVE/ACT.
              pWH = [
                  ps.tile([HH, 2, SB], f32, tag=f"pWH_{s}", name=f"pWH_{s}")
                  for s in ("A", "B")
              ]
              pWO = [
                  ps.tile([HH, 2, SB], f32, tag=f"pWO_{s}", name=f"pWO_{s}")
                  for s in ("A", "B")
              ]
              zacc = ps.tile([HH, 4, SB], f32, tag="zacc", name="zacc")

              # Software-pipelined recurrence: the A and B batch sub-blocks
              # are independent chains; B's stages are emitted half an
              # iteration behind A so each chain's cross-engine handoff
              # latency is covered by the other chain's work.  Identity
              # matmuls (ZACC accumulate) lag behind the next wh-stage so
              # the PE never waits on a dtanh-multiply.
              def stage1_mms(s, it, curr):
                  cSl = slice(2 * s, 2 * s + 2)
                  for m in range(2):
                      mSl = slice(m * HH, (m + 1) * HH)
                      if it == 0:
                          nc.tensor.matmul(
                              pWH[s][:, m, :], wx_b[:, mSl], xd_b[:, s, :],
                              start=True, stop=True,
                          )
                      else:
                          nc.tensor.matmul(
                              pWH[s][:, m, :], wh_b[0][:, mSl],
                              curr[:, 2 * s, :], start=True, stop=False,
                          )
                          nc.tensor.matmul(
                              pWH[s][:, m, :], wh_b[1][:, mSl],
                              curr[:, 2 * s + 1, :], start=False, stop=True,
                          )

              def gate(s, tg):
                  cSl = slice(2 * s, 2 * s + 2)
                  nc.vector.scalar_tensor_tensor(
                      tg[:, cSl, :], relu_arr[:, cSl, :], 0.0,
                      pWH[s][:], _ALU.is_gt, _ALU.mult,
                  )

              def wout_mms(s, tg):
                  for m in range(2):
                      mSl = slice(m * HH, (m + 1) * HH)
                      nc.tensor.matmul(
                          pWO[s][:, m, :], wo_b[0][:, mSl],
                          tg[:, 2 * s, :], start=True, stop=False,
                      )
                      nc.tensor.matmul(
                          pWO[s][:, m, :], wo_b[1][:, mSl],
                          tg[:, 2 * s + 1, :], start=False, stop=True,
                      )

              def mul(s, newc):
                  cSl = slice(2 * s, 2 * s + 2)
                  nc.vector.tensor_mul(
                      newc[:, cSl, :], dtanh_arr[:, cSl, :], pWO[s][:]
                  )

              def id_mm(s, it, newc):
                  nc.tensor.matmul(
                      zacc[:, 2 * s : 2 * s + 2, :], id_b[:, :],
                      newc[:, 2 * s : 2 * s + 2, :],
                      start=(it == 0 and s == 0), stop=(it == K_TERMS and s == 1),
                      skip_group_check=True,
                  )

              curr = None
              pend = []  # (s, it, tile) id-mms not yet emitted
              tiles = {}
              for it in range(K_TERMS + 1):
                  tiles[it] = (
                      rot.tile([HH, 4, SB], bf16, tag="tg", name="tg"),
                      rot.tile([HH, 4, SB], bf16, tag="curr", name="newc"),
                  )
                  tg, newc = tiles[it]
                  for s in range(2):
                      stage1_mms(s, it, curr)
                      gate(s, tg)
                      wout_mms(s, tg)
                      while pend:
                          id_mm(*pend.pop(0))
                      mul(s, newc)
                      pend.append((s, it, newc))
                  curr = newc
              while pend:
                  id_mm(*pend.pop(0))

              # ---- tail: drain ZACC (split DVE/ACT) and store (2 DMAs) ----
              hdot_sb = apool.tile([HH, 4, SB], f32, tag="hd", name="hdot_sb")
              nc.vector.tensor_copy(hdot_sb[:, 0::2, :], zacc[:, 0::2, :])
              nc.scalar.copy(hdot_sb[:, 1::2, :], zacc[:, 1::2, :])
              nc.sync.dma_start(hdT[0:HH, :], hdot_sb[:, 0::2, :])
              nc.scalar.dma_start(hdT[HH:H, :], hdot_sb[:, 1::2, :])

    nc.compile()
    return nc


_NC = {}


def _get_nc(repeat=1, loop=0):
    key = (repeat, loop)
    if key not in _NC:
        _NC[key] = _build(repeat, loop)
    return _NC[key]


def make_in_maps(h, x, xdot, wx, wh, wout, b0, b1):
    h = np.asarray(h, np.float32)
    x = np.asarray(x, np.float32)
    xdot = np.asarray(xdot, np.float32)
    wx = np.asarray(wx, np.float32)
    wh = np.asarray(wh, np.float32)
    wout = np.asarray(wout, np.float32)
    b0 = np.asarray(b0, np.float32)
    b1 = np.asarray(b1, np.float32)

    whT = np.ascontiguousarray(wh.T)
    woT = np.ascontiguousarray(wout.T)
    wxT = np.ascontiguousarray(wx.T)
    whB = whT.astype(ml_dtypes.bfloat16)
    woB = woT.astype(ml_dtypes.bfloat16)
    wxB = wxT.astype(ml_dtypes.bfloat16)
    idB = np.eye(HH, dtype=np.float32).astype(ml_dtypes.bfloat16)
    b0c = np.ascontiguousarray(np.stack([b0[:HH], b0[HH:]], axis=1))
    b1c = np.ascontiguousarray(np.stack([b1[:HH], b1[HH:]], axis=1))

    u8 = np.uint8
    w_part = [whT[:HH].view(u8), whT[HH:].view(u8)]
    blobB = np.concatenate(
        [whB[:HH].view(u8), whB[HH:].view(u8),
         woB[:HH].view(u8), woB[HH:].view(u8), idB.view(u8)], axis=1
    )
    bias_blob = np.concatenate([b0c.view(u8), b1c.view(u8)], axis=1)

    in_maps = []
    for i in range(N_CORES):
        sl = slice(i * BL, (i + 1) * BL)
        hT = np.ascontiguousarray(h[sl].T)
        xT = np.ascontiguousarray(x[sl].T)
        xdB = np.ascontiguousarray(xdot[sl].T).astype(ml_dtypes.bfloat16)
        blobA1 = np.concatenate([w_part[0], hT[:HH].view(u8)], axis=1)
        blobA2 = np.concatenate([w_part[1], hT[HH:].view(u8)], axis=1)
        blob64 = np.concatenate(
            [wxT.view(u8), xT.view(u8), wxB.view(u8), xdB.view(u8)], axis=1
        )
        in_maps.append(
            {
                "blobA1": blobA1,
                "blobA2": blobA2,
                "blobB": blobB,
                "blob64": blob64,
                "biasb": bias_blob,
            }
        )
    return in_maps


def kernel(h, x, xdot, wx, wh, wout, b0, b1):
    in_maps = make_in_maps(h, x, xdot, wx, wh, wout, b0, b1)
    res = run_bass_kernel_spmd(_get_nc(), in_maps, core_ids=list(range(N_CORES)))
    out = np.empty((B, H), np.float32)
    for i in range(N_CORES):
        out[i * BL : (i + 1) * BL] = res.results[i]["hdT"].T
    return out
